# revision 23
# baseline (speedup 1.0000x reference)
"""Trainium2 Bass kernel for nn_GetNodeK (gnn_message_passing).

out[b,i,n,m,:] = node_embedding[b, nbr_idx[b, nbr_idx[b,i,n], m], :]

Sharding: data-parallel over B (8 batches -> 8 cores, one batch per core).

Let nbr_flat = nbr_idx[b].reshape(6144) (values < 256) and define the
one-hop table G[j] = concat_m emb[nbr[j,m]] (256 rows x 12 KB = 3.1 MB).
Then out[b, k=(i*24+n)] = G[nbr_flat[k]] -- the 2-hop gather factors into
two index-driven stages that both use the raw nbr values (no chained
index arithmetic anywhere).

v9 (default, ~225 us vs 496 us baseline): raw bass (no TileContext).
G is built by TensorE permutation matmuls from host-uploaded exact fp16
one-hot matrices (PSUM f32, exactly one nonzero term per output -> the
only error is fp16 rounding of emb, rel err ~5e-3 << 2e-2); DVE copies
PSUM->SBUF.  The Pool engine runs ONLY the 2T [128,1]-offset indirect
scatter rounds back-to-back (manual semaphores, no conservative WAW deps
between rounds), so the 16 SDMA engines stream the 75.5 MB output write
at their ~390 GB/s aggregate line rate with no serialization.

Earlier iterations kept as fallback: v2 (tile, serialized rounds,
~496 us), v4 (raw bass + dma_gather stage 1, ~276 us), v6/v7 (v5 with
chunked pt loads / 6 KB row-piece scatters -- measured no better).
"""
import numpy as np

from concourse import bass, bacc, mybir
import concourse.tile as tile
from concourse.bass_utils import run_bass_kernel_spmd

B, At, Nbr, F = 8, 256, 24, 128
NI = At * Nbr        # 6144 indices per batch
ROW = Nbr * F        # 3072 f32 = 12 KB per stage-2 row
HALF = NI // 2       # 3072 gather indices per token-half
OOB = 8192           # idx sentinel > NI-1 -> skipped by bounds_check

VERSION = "v10"
_CACHED = {}


_T_PERM = None


def _v1_perm():
    """idx1[t] = nbr[(t//128//24)*128 + t%128, (t//128)%24] as flat index."""
    global _T_PERM
    if _T_PERM is None:
        t = np.arange(NI)
        s, p = t // 128, t % 128
        j, m = (s // Nbr) * 128 + p, s % Nbr
        _T_PERM = j * Nbr + m
    return _T_PERM


def _prep_gidx(nbr16_b):
    idx1 = nbr16_b.reshape(-1)[_v1_perm()]
    return np.tile(idx1.reshape(NI // 16, 16).T, (8, 1))


def _occurrence_tbl(flat, T):
    """tbl[j, t] = flat position of the t-th occurrence of token j."""
    counts = np.bincount(flat, minlength=At)
    order = np.argsort(flat, kind="stable")
    tbl = np.full((At, T), OOB, dtype=np.int32)
    pos = 0
    for j in range(At):
        c = counts[j]
        tbl[j, :c] = order[pos:pos + c]
        pos += c
    return tbl


# ---------------------------------------------------------------- v3 ----
def _prep_v3(nbr16_b, T):
    tbl = _occurrence_tbl(nbr16_b.reshape(-1), T)
    sidx = np.ascontiguousarray(tbl.reshape(2, 128, T).transpose(1, 0, 2))
    return {"gidx": _prep_gidx(nbr16_b), "sidx": sidx}


def _build_nc_v3(T):
    nc = bacc.Bacc("TRN2", target_bir_lowering=False, debug=False)
    emb = nc.dram_tensor("emb", [At, F], mybir.dt.float32, kind="ExternalInput")
    gidx = nc.dram_tensor("gidx", [128, NI // 16], mybir.dt.int16, kind="ExternalInput")
    sidx = nc.dram_tensor("sidx", [128, 2, T], mybir.dt.int32, kind="ExternalInput")
    out = nc.dram_tensor("out", [NI, ROW], mybir.dt.float32, kind="ExternalOutput")

    with tile.TileContext(nc) as tc:
        with tc.tile_pool(name="pool0", bufs=1) as pool0:
            idx_t = pool0.tile([128, NI // 16], mybir.dt.int16)
            nc.sync.dma_start(idx_t[:], gidx[:])
            sidx_t = pool0.tile([128, 2, T], mybir.dt.int32)
            nc.sync.dma_start(sidx_t[:], sidx[:])

            g_t = pool0.tile([128, NI // 128, F], mybir.dt.float32)
            g_scatter = g_t[:].rearrange("p (q m) e -> p q (m e)", q=2)
            for q in range(2):
                nc.gpsimd.dma_gather(
                    g_t[:, q * Nbr:(q + 1) * Nbr, :], emb[:],
                    idx_t[:, q * (HALF // 16):(q + 1) * (HALF // 16)],
                    HALF, HALF, F, single_packet=False,
                )
                src = g_scatter[:, q, :].unsqueeze(1).to_broadcast([128, T, ROW])
                nc.gpsimd.indirect_dma_start(
                    out=out[:],
                    out_offset=bass.IndirectOffsetOnAxis(
                        ap=sidx_t[:, q, :], axis=0),
                    in_=src,
                    in_offset=None,
                    bounds_check=NI - 1,
                    oob_is_err=False,
                )
    nc.compile()
    return nc


# ---------------------------------------------------------------- v4 ----
# Raw bass (no TileContext): identical per-round [128,1]-offset scatters as
# v2, but without Tile's conservative WAW deps between rounds -- the Pool
# engine issues all 2T descriptor-generation ops back-to-back and the 16
# SDMA engines drain continuously.  Sync is manual: idx loads -> gather
# half -> that half's T scatter rounds; final wait on the scatter sem.
def _prep_v4(nbr16_b, T):
    return _prep_v2(nbr16_b, T)


def _build_nc_v4(T):
    nc = bacc.Bacc("TRN2", target_bir_lowering=False, debug=False)
    emb = nc.dram_tensor("emb", [At, F], mybir.dt.float32, kind="ExternalInput")
    gidx = nc.dram_tensor("gidx", [128, NI // 16], mybir.dt.int16, kind="ExternalInput")
    sidx = nc.dram_tensor("sidx", [128, T, 2], mybir.dt.int32, kind="ExternalInput")
    out = nc.dram_tensor("out", [NI, ROW], mybir.dt.float32, kind="ExternalOutput")

    idx_t = nc.alloc_sbuf_tensor("idx_t", [128, NI // 16], mybir.dt.int16)
    sidx_t = nc.alloc_sbuf_tensor("sidx_t", [128, T, 2], mybir.dt.int32)
    g_t = nc.alloc_sbuf_tensor("g_t", [128, NI // 128, F], mybir.dt.float32)

    sem_idx = nc.alloc_semaphore("sem_idx")
    sem_g = nc.alloc_semaphore("sem_g")
    sem_out = nc.alloc_semaphore("sem_out")

    with nc.Block() as blk:

        @blk.sync
        def _(sync):
            sync.dma_start(idx_t[:], gidx[:]).then_inc(sem_idx, 16)
            sync.dma_start(sidx_t[:], sidx[:]).then_inc(sem_idx, 16)

        @blk.gpsimd
        def _(g):
            g.wait_ge(sem_idx, 32)
            g_scatter = g_t[:].rearrange("p (q m) e -> p q (m e)", q=2)
            for q in range(2):
                g.dma_gather(
                    g_t[:, q * Nbr:(q + 1) * Nbr, :], emb[:],
                    idx_t[:, q * (HALF // 16):(q + 1) * (HALF // 16)],
                    HALF, HALF, F, single_packet=False,
                ).then_inc(sem_g, 16)
                g.wait_ge(sem_g, 16 * (q + 1))
                for r in range(T):
                    g.indirect_dma_start(
                        out=out[:],
                        out_offset=bass.IndirectOffsetOnAxis(
                            ap=sidx_t[:, r, q:q + 1], axis=0),
                        in_=g_scatter[:, q, :],
                        in_offset=None,
                        bounds_check=NI - 1,
                        oob_is_err=False,
                    ).then_inc(sem_out, 16)
            g.wait_ge(sem_out, 16 * 2 * T)

    nc.compile()
    return nc


# ---------------------------------------------------------------- v5 ----
# v4 + stage-1 gather moved off the GpSimd/DMA path entirely: G is built by
# TensorE permutation matmuls.  Host uploads exact fp16 one-hot matrices
# PT[(q*24+m)*2+h][i, j] = (nbr[q*128+j, m] == h*128+i); per (q,m) tile
# G[j, :] = PT_lo.T @ emb_lo + PT_hi.T @ emb_hi accumulates in PSUM (f32,
# exactly one nonzero term -> result is just emb rounded to fp16, rel err
# ~2^-11 << 2e-2 gate).  DVE copies PSUM->SBUF.  The Pool engine runs ONLY
# the 2T indirect-scatter rounds, and the 16 SDMA engines carry nothing but
# the 75.5 MB output write.
NT = 2 * Nbr         # 48 (q,m) tiles
NG = NT // 4         # 12 groups of 4 tiles (one PSUM bank each)


def _prep_v5(nbr16_b, T):
    nbr_r = nbr16_b.reshape(2, 128, Nbr).astype(np.int64)  # [q, j, m]
    pt = np.zeros((128, 2, Nbr, 2, 128), dtype=np.float16)  # [i, q, m, h, j]
    q_ix, j_ix, m_ix = np.meshgrid(np.arange(2), np.arange(128),
                                   np.arange(Nbr), indexing="ij")
    vals = nbr_r[q_ix, j_ix, m_ix]
    pt[vals % 128, q_ix, m_ix, vals // 128, j_ix] = np.float16(1.0)
    ptd = np.ascontiguousarray(pt.reshape(128, NT * 2, 128))

    tbl = _occurrence_tbl(nbr16_b.reshape(-1), T)
    sidx = np.empty((128, T, 2), dtype=np.int32)
    for q in range(2):
        sidx[:, :, q] = tbl[q * 128:(q + 1) * 128, :]
    return {"ptd": ptd, "sidx": sidx}


def _build_nc_v5(T):
    nc = bacc.Bacc("TRN2", target_bir_lowering=False, debug=False)
    emb16d = nc.dram_tensor("emb16", [128, 2, F], mybir.dt.float16, kind="ExternalInput")
    ptd = nc.dram_tensor("ptd", [128, NT * 2, 128], mybir.dt.float16, kind="ExternalInput")
    sidxd = nc.dram_tensor("sidx", [128, T, 2], mybir.dt.int32, kind="ExternalInput")
    out = nc.dram_tensor("out", [NI, ROW], mybir.dt.float32, kind="ExternalOutput")

    emb_t = nc.alloc_sbuf_tensor("emb_t", [128, 2, F], mybir.dt.float16)
    pt_t = nc.alloc_sbuf_tensor("pt_t", [128, NT * 2, 128], mybir.dt.float16)
    sidx_t = nc.alloc_sbuf_tensor("sidx_t", [128, T, 2], mybir.dt.int32)
    g_t = nc.alloc_sbuf_tensor("g_t", [128, NI // 128, F], mybir.dt.float32)
    ps = nc.alloc_psum_tensor("ps", [128, 8, 128], mybir.dt.float32)

    sem_in = nc.alloc_semaphore("sem_in")
    sem_in2 = nc.alloc_semaphore("sem_in2")
    sem_sidx = nc.alloc_semaphore("sem_sidx")
    sem_pe = nc.alloc_semaphore("sem_pe")
    sem_dve = nc.alloc_semaphore("sem_dve")
    sem_out = nc.alloc_semaphore("sem_out")

    with nc.Block() as blk:

        @blk.sync
        def _(sync):
            sync.dma_start(emb_t[:], emb16d[:]).then_inc(sem_in, 16)
            # pt halves separately so PE can start on half 0 sooner
            sync.dma_start(pt_t[:, :NT, :], ptd[:, :NT, :]).then_inc(sem_in, 16)
            sync.dma_start(pt_t[:, NT:, :], ptd[:, NT:, :]).then_inc(sem_in2, 16)
            sync.dma_start(sidx_t[:], sidxd[:]).then_inc(sem_sidx, 16)

        @blk.tensor
        def _(te):
            te.wait_ge(sem_in, 32)  # emb + pt half 0
            for g in range(NG):
                if g == NG // 2:
                    te.wait_ge(sem_in2, 16)  # pt half 1
                if g >= 2:
                    te.wait_ge(sem_dve, g - 1)  # bank g%2 reusable
                bank = g % 2
                for k in range(4):
                    s = 4 * g + k
                    te.matmul(out=ps[:, 4 * bank + k, :],
                              lhsT=pt_t[:, 2 * s, :], rhs=emb_t[:, 0, :],
                              start=True, stop=False)
                    mm = te.matmul(out=ps[:, 4 * bank + k, :],
                                   lhsT=pt_t[:, 2 * s + 1, :], rhs=emb_t[:, 1, :],
                                   start=False, stop=True)
                    if k == 3:
                        mm.then_inc(sem_pe, 1)

        @blk.vector
        def _(ve):
            for g in range(NG):
                ve.wait_ge(sem_pe, g + 1)
                bank = g % 2
                ve.tensor_copy(
                    out=g_t[:, 4 * g:4 * g + 4, :],
                    in_=ps[:, 4 * bank:4 * bank + 4, :],
                ).then_inc(sem_dve, 1)

        @blk.gpsimd
        def _(g):
            g_scatter = g_t[:].rearrange("p (q m) e -> p q (m e)", q=2)
            g.wait_ge(sem_sidx, 16)
            for q in range(2):
                g.wait_ge(sem_dve, (NG // 2) * (q + 1))
                for r in range(T):
                    g.indirect_dma_start(
                        out=out[:],
                        out_offset=bass.IndirectOffsetOnAxis(
                            ap=sidx_t[:, r, q:q + 1], axis=0),
                        in_=g_scatter[:, q, :],
                        in_offset=None,
                        bounds_check=NI - 1,
                        oob_is_err=False,
                    ).then_inc(sem_out, 16)
            g.wait_ge(sem_out, 16 * 2 * T)

    nc.compile()
    return nc


# ---------------------------------------------------------------- v6 ----
# v5 with the pt upload split into 4 chunks (own semaphores, FIFO HWDGE)
# so the PE can start building G as soon as the first 12 tiles land,
# pulling the first scatter round ~5us earlier.
def _prep_v6(nbr16_b, T):
    return _prep_v5(nbr16_b, T)


def _build_nc_v6(T):
    nc = bacc.Bacc("TRN2", target_bir_lowering=False, debug=False)
    emb16d = nc.dram_tensor("emb16", [128, 2, F], mybir.dt.float16, kind="ExternalInput")
    ptd = nc.dram_tensor("ptd", [128, NT * 2, 128], mybir.dt.float16, kind="ExternalInput")
    sidxd = nc.dram_tensor("sidx", [128, T, 2], mybir.dt.int32, kind="ExternalInput")
    out = nc.dram_tensor("out", [NI, ROW], mybir.dt.float32, kind="ExternalOutput")

    emb_t = nc.alloc_sbuf_tensor("emb_t", [128, 2, F], mybir.dt.float16)
    pt_t = nc.alloc_sbuf_tensor("pt_t", [128, NT * 2, 128], mybir.dt.float16)
    sidx_t = nc.alloc_sbuf_tensor("sidx_t", [128, T, 2], mybir.dt.int32)
    g_t = nc.alloc_sbuf_tensor("g_t", [128, NI // 128, F], mybir.dt.float32)
    ps = nc.alloc_psum_tensor("ps", [128, 8, 128], mybir.dt.float32)

    sem_emb = nc.alloc_semaphore("sem_emb")
    sem_pt = [nc.alloc_semaphore(f"sem_pt{c}") for c in range(4)]
    sem_sidx = nc.alloc_semaphore("sem_sidx")
    sem_pe = nc.alloc_semaphore("sem_pe")
    sem_dve = nc.alloc_semaphore("sem_dve")
    sem_out = nc.alloc_semaphore("sem_out")

    CH = NT * 2 // 4  # 24 pt tiles per chunk = 3 groups

    with nc.Block() as blk:

        @blk.sync
        def _(sync):
            sync.dma_start(emb_t[:], emb16d[:]).then_inc(sem_emb, 16)
            for c in range(4):
                sync.dma_start(pt_t[:, c * CH:(c + 1) * CH, :],
                               ptd[:, c * CH:(c + 1) * CH, :]).then_inc(sem_pt[c], 16)
            sync.dma_start(sidx_t[:], sidxd[:]).then_inc(sem_sidx, 16)

        @blk.tensor
        def _(te):
            te.wait_ge(sem_emb, 16)
            for g in range(NG):
                if g % 3 == 0:
                    te.wait_ge(sem_pt[g // 3], 16)
                if g >= 2:
                    te.wait_ge(sem_dve, g - 1)
                bank = g % 2
                for k in range(4):
                    s = 4 * g + k
                    te.matmul(out=ps[:, 4 * bank + k, :],
                              lhsT=pt_t[:, 2 * s, :], rhs=emb_t[:, 0, :],
                              start=True, stop=False)
                    mm = te.matmul(out=ps[:, 4 * bank + k, :],
                                   lhsT=pt_t[:, 2 * s + 1, :], rhs=emb_t[:, 1, :],
                                   start=False, stop=True)
                    if k == 3:
                        mm.then_inc(sem_pe, 1)

        @blk.vector
        def _(ve):
            for g in range(NG):
                ve.wait_ge(sem_pe, g + 1)
                bank = g % 2
                ve.tensor_copy(
                    out=g_t[:, 4 * g:4 * g + 4, :],
                    in_=ps[:, 4 * bank:4 * bank + 4, :],
                ).then_inc(sem_dve, 1)

        @blk.gpsimd
        def _(g):
            g_scatter = g_t[:].rearrange("p (q m) e -> p q (m e)", q=2)
            g.wait_ge(sem_sidx, 16)
            for q in range(2):
                g.wait_ge(sem_dve, (NG // 2) * (q + 1))
                for r in range(T):
                    g.indirect_dma_start(
                        out=out[:],
                        out_offset=bass.IndirectOffsetOnAxis(
                            ap=sidx_t[:, r, q:q + 1], axis=0),
                        in_=g_scatter[:, q, :],
                        in_offset=None,
                        bounds_check=NI - 1,
                        oob_is_err=False,
                    ).then_inc(sem_out, 16)
            g.wait_ge(sem_out, 16 * 2 * T)

    nc.compile()
    return nc


# ---------------------------------------------------------------- v7 ----
# v6 + earlier drain start: half 0 is scattered as two 6 KB row-pieces
# against a [2*NI, ROW/2] view of out (row 2k+h = columns [h*1536,(h+1)*1536)
# of out row k -- same memory, offset 0, so no element_offset needed).
# Piece 0 only needs m-slots 0..11 (PE groups 0-2), pulling the first
# scatter ~5us earlier; half 1 stays full-row 12 KB.
def _prep_v7(nbr16_b, T):
    d = _prep_v5(nbr16_b, T)
    tbl = _occurrence_tbl(nbr16_b.reshape(-1), T)  # [At, T], OOB-padded
    h0 = tbl[:128, :]                              # tokens 0..127
    sidx2 = np.empty((128, T, 2), dtype=np.int32)  # [p, r, h] -> 2*row+h
    for h in range(2):
        sidx2[:, :, h] = 2 * h0 + h
    d["sidx2"] = sidx2
    return d


def _build_nc_v7(T):
    nc = bacc.Bacc("TRN2", target_bir_lowering=False, debug=False)
    emb16d = nc.dram_tensor("emb16", [128, 2, F], mybir.dt.float16, kind="ExternalInput")
    ptd = nc.dram_tensor("ptd", [128, NT * 2, 128], mybir.dt.float16, kind="ExternalInput")
    sidxd = nc.dram_tensor("sidx", [128, T, 2], mybir.dt.int32, kind="ExternalInput")
    sidx2d = nc.dram_tensor("sidx2", [128, T, 2], mybir.dt.int32, kind="ExternalInput")
    out = nc.dram_tensor("out", [NI, ROW], mybir.dt.float32, kind="ExternalOutput")

    emb_t = nc.alloc_sbuf_tensor("emb_t", [128, 2, F], mybir.dt.float16)
    pt_t = nc.alloc_sbuf_tensor("pt_t", [128, NT * 2, 128], mybir.dt.float16)
    sidx_t = nc.alloc_sbuf_tensor("sidx_t", [128, T, 2], mybir.dt.int32)
    sidx2_t = nc.alloc_sbuf_tensor("sidx2_t", [128, T, 2], mybir.dt.int32)
    g_t = nc.alloc_sbuf_tensor("g_t", [128, NI // 128, F], mybir.dt.float32)
    ps = nc.alloc_psum_tensor("ps", [128, 8, 128], mybir.dt.float32)

    sem_emb = nc.alloc_semaphore("sem_emb")
    sem_pt = [nc.alloc_semaphore(f"sem_pt{c}") for c in range(4)]
    sem_sidx = nc.alloc_semaphore("sem_sidx")
    sem_pe = nc.alloc_semaphore("sem_pe")
    sem_dve = nc.alloc_semaphore("sem_dve")
    sem_out = nc.alloc_semaphore("sem_out")

    CH = NT * 2 // 4

    with nc.Block() as blk:

        @blk.sync
        def _(sync):
            sync.dma_start(emb_t[:], emb16d[:]).then_inc(sem_emb, 16)
            for c in range(4):
                sync.dma_start(pt_t[:, c * CH:(c + 1) * CH, :],
                               ptd[:, c * CH:(c + 1) * CH, :]).then_inc(sem_pt[c], 16)
            sync.dma_start(sidx_t[:], sidxd[:]).then_inc(sem_sidx, 16)
            sync.dma_start(sidx2_t[:], sidx2d[:]).then_inc(sem_sidx, 16)

        @blk.tensor
        def _(te):
            te.wait_ge(sem_emb, 16)
            for g in range(NG):
                if g % 3 == 0:
                    te.wait_ge(sem_pt[g // 3], 16)
                if g >= 2:
                    te.wait_ge(sem_dve, g - 1)
                bank = g % 2
                for k in range(4):
                    s = 4 * g + k
                    te.matmul(out=ps[:, 4 * bank + k, :],
                              lhsT=pt_t[:, 2 * s, :], rhs=emb_t[:, 0, :],
                              start=True, stop=False)
                    mm = te.matmul(out=ps[:, 4 * bank + k, :],
                                   lhsT=pt_t[:, 2 * s + 1, :], rhs=emb_t[:, 1, :],
                                   start=False, stop=True)
                    if k == 3:
                        mm.then_inc(sem_pe, 1)

        @blk.vector
        def _(ve):
            for g in range(NG):
                ve.wait_ge(sem_pe, g + 1)
                bank = g % 2
                ve.tensor_copy(
                    out=g_t[:, 4 * g:4 * g + 4, :],
                    in_=ps[:, 4 * bank:4 * bank + 4, :],
                ).then_inc(sem_dve, 1)

        @blk.gpsimd
        def _(g):
            out2 = out[:].rearrange("k (h e) -> (k h) e", h=2)  # [2*NI, 1536]
            g_scatter = g_t[:].rearrange("p (q m) e -> p q (m e)", q=2)
            g_half = g_t[:].rearrange("p (x y) e -> p x (y e)", x=4)  # 6KB quarters
            g.wait_ge(sem_sidx, 32)
            nrounds = 0
            # half 0 as two 6KB pieces (piece h needs PE groups 0-2 / 3-5)
            for h in range(2):
                g.wait_ge(sem_dve, 3 * (h + 1))
                for r in range(T):
                    g.indirect_dma_start(
                        out=out2,
                        out_offset=bass.IndirectOffsetOnAxis(
                            ap=sidx2_t[:, r, h:h + 1], axis=0),
                        in_=g_half[:, h, :],
                        in_offset=None,
                        bounds_check=2 * NI - 1,
                        oob_is_err=False,
                    ).then_inc(sem_out, 16)
                    nrounds += 1
            # half 1 full 12KB rows
            g.wait_ge(sem_dve, NG)
            for r in range(T):
                g.indirect_dma_start(
                    out=out[:],
                    out_offset=bass.IndirectOffsetOnAxis(
                        ap=sidx_t[:, r, 1:2], axis=0),
                    in_=g_scatter[:, 1, :],
                    in_offset=None,
                    bounds_check=NI - 1,
                    oob_is_err=False,
                ).then_inc(sem_out, 16)
                nrounds += 1
            g.wait_ge(sem_out, 16 * nrounds)

    nc.compile()
    return nc


# ---------------------------------------------------------------- v8 ----
# v5 with: (a) PT one-hots in fp8e4 (0/1 exact; halves the 3MB upload that
# gates the PE start), (b) pt half 0 loaded before emb, (c) the block exit
# skips GpSimd's dge_drain (no_gpsimd_drain=True) -- the explicit sem_out
# wait already guarantees every output byte landed.
def _prep_v8(nbr16_b, T):
    d = _prep_v5(nbr16_b, T)
    d["ptd"] = d["ptd"].astype(mybir.dt.np(mybir.dt.float8e4))
    return d


def _build_nc_v8(T):
    nc = bacc.Bacc("TRN2", target_bir_lowering=False, debug=False)
    emb16d = nc.dram_tensor("emb16", [128, 2, F], mybir.dt.float16, kind="ExternalInput")
    ptd = nc.dram_tensor("ptd", [128, NT * 2, 128], mybir.dt.float8e4, kind="ExternalInput")
    sidxd = nc.dram_tensor("sidx", [128, T, 2], mybir.dt.int32, kind="ExternalInput")
    out = nc.dram_tensor("out", [NI, ROW], mybir.dt.float32, kind="ExternalOutput")

    emb_t = nc.alloc_sbuf_tensor("emb_t", [128, 2, F], mybir.dt.float16)
    pt_t = nc.alloc_sbuf_tensor("pt_t", [128, NT * 2, 128], mybir.dt.float8e4)
    sidx_t = nc.alloc_sbuf_tensor("sidx_t", [128, T, 2], mybir.dt.int32)
    g_t = nc.alloc_sbuf_tensor("g_t", [128, NI // 128, F], mybir.dt.float32)
    ps = nc.alloc_psum_tensor("ps", [128, 8, 128], mybir.dt.float32)

    sem_emb = nc.alloc_semaphore("sem_emb")
    sem_pt0 = nc.alloc_semaphore("sem_pt0")
    sem_pt1 = nc.alloc_semaphore("sem_pt1")
    sem_sidx = nc.alloc_semaphore("sem_sidx")
    sem_pe = nc.alloc_semaphore("sem_pe")
    sem_dve = nc.alloc_semaphore("sem_dve")
    sem_out = nc.alloc_semaphore("sem_out")

    with nc.Block(no_gpsimd_drain=True) as blk:

        @blk.sync
        def _(sync):
            sync.dma_start(pt_t[:, :NT, :], ptd[:, :NT, :]).then_inc(sem_pt0, 16)
            sync.dma_start(emb_t[:], emb16d[:]).then_inc(sem_emb, 16)
            sync.dma_start(pt_t[:, NT:, :], ptd[:, NT:, :]).then_inc(sem_pt1, 16)
            sync.dma_start(sidx_t[:], sidxd[:]).then_inc(sem_sidx, 16)

        @blk.tensor
        def _(te):
            te.wait_ge(sem_pt0, 16)
            te.wait_ge(sem_emb, 16)
            for g in range(NG):
                if g == NG // 2:
                    te.wait_ge(sem_pt1, 16)
                if g >= 2:
                    te.wait_ge(sem_dve, g - 1)
                bank = g % 2
                for k in range(4):
                    s = 4 * g + k
                    te.matmul(out=ps[:, 4 * bank + k, :],
                              lhsT=pt_t[:, 2 * s, :], rhs=emb_t[:, 0, :],
                              start=True, stop=False)
                    mm = te.matmul(out=ps[:, 4 * bank + k, :],
                                   lhsT=pt_t[:, 2 * s + 1, :], rhs=emb_t[:, 1, :],
                                   start=False, stop=True)
                    if k == 3:
                        mm.then_inc(sem_pe, 1)

        @blk.vector
        def _(ve):
            for g in range(NG):
                ve.wait_ge(sem_pe, g + 1)
                bank = g % 2
                ve.tensor_copy(
                    out=g_t[:, 4 * g:4 * g + 4, :],
                    in_=ps[:, 4 * bank:4 * bank + 4, :],
                ).then_inc(sem_dve, 1)

        @blk.gpsimd
        def _(g):
            g_scatter = g_t[:].rearrange("p (q m) e -> p q (m e)", q=2)
            g.wait_ge(sem_sidx, 16)
            for q in range(2):
                g.wait_ge(sem_dve, (NG // 2) * (q + 1))
                for r in range(T):
                    g.indirect_dma_start(
                        out=out[:],
                        out_offset=bass.IndirectOffsetOnAxis(
                            ap=sidx_t[:, r, q:q + 1], axis=0),
                        in_=g_scatter[:, q, :],
                        in_offset=None,
                        bounds_check=NI - 1,
                        oob_is_err=False,
                    ).then_inc(sem_out, 16)
            g.wait_ge(sem_out, 16 * 2 * T)

    nc.compile()
    return nc


# ---------------------------------------------------------------- v9 ----
# v8 with the pt upload split into 6 chunks of 16 tiles (2 PE groups each)
# so matmuls stream right behind the DMA instead of waiting for the full
# 0.75 MB half.
def _prep_v9(nbr16_b, T):
    return _prep_v8(nbr16_b, T)


def _build_nc_v9(T):
    nc = bacc.Bacc("TRN2", target_bir_lowering=False, debug=False)
    emb16d = nc.dram_tensor("emb16", [128, 2, F], mybir.dt.float16, kind="ExternalInput")
    ptd = nc.dram_tensor("ptd", [128, NT * 2, 128], mybir.dt.float8e4, kind="ExternalInput")
    sidxd = nc.dram_tensor("sidx", [128, T, 2], mybir.dt.int32, kind="ExternalInput")
    out = nc.dram_tensor("out", [NI, ROW], mybir.dt.float32, kind="ExternalOutput")

    emb_t = nc.alloc_sbuf_tensor("emb_t", [128, 2, F], mybir.dt.float16)
    pt_t = nc.alloc_sbuf_tensor("pt_t", [128, NT * 2, 128], mybir.dt.float8e4)
    sidx_t = nc.alloc_sbuf_tensor("sidx_t", [128, T, 2], mybir.dt.int32)
    g_t = nc.alloc_sbuf_tensor("g_t", [128, NI // 128, F], mybir.dt.float32)
    ps = nc.alloc_psum_tensor("ps", [128, 8, 128], mybir.dt.float32)

    sem_emb = nc.alloc_semaphore("sem_emb")
    sem_pt = [nc.alloc_semaphore(f"sem_pt{c}") for c in range(6)]
    sem_sidx = nc.alloc_semaphore("sem_sidx")
    sem_pe = nc.alloc_semaphore("sem_pe")
    sem_dve = nc.alloc_semaphore("sem_dve")
    sem_out = nc.alloc_semaphore("sem_out")

    CH = NT * 2 // 6  # 16 pt tiles per chunk = 2 PE groups

    with nc.Block(no_gpsimd_drain=True) as blk:

        @blk.sync
        def _(sync):
            sync.dma_start(emb_t[:], emb16d[:]).then_inc(sem_emb, 16)
            for c in range(6):
                sync.dma_start(pt_t[:, c * CH:(c + 1) * CH, :],
                               ptd[:, c * CH:(c + 1) * CH, :]).then_inc(sem_pt[c], 16)
            sync.dma_start(sidx_t[:], sidxd[:]).then_inc(sem_sidx, 16)

        @blk.tensor
        def _(te):
            te.wait_ge(sem_emb, 16)
            for g in range(NG):
                if g % 2 == 0:
                    te.wait_ge(sem_pt[g // 2], 16)
                if g >= 2:
                    te.wait_ge(sem_dve, g - 1)
                bank = g % 2
                for k in range(4):
                    s = 4 * g + k
                    te.matmul(out=ps[:, 4 * bank + k, :],
                              lhsT=pt_t[:, 2 * s, :], rhs=emb_t[:, 0, :],
                              start=True, stop=False)
                    mm = te.matmul(out=ps[:, 4 * bank + k, :],
                                   lhsT=pt_t[:, 2 * s + 1, :], rhs=emb_t[:, 1, :],
                                   start=False, stop=True)
                    if k == 3:
                        mm.then_inc(sem_pe, 1)

        @blk.vector
        def _(ve):
            for g in range(NG):
                ve.wait_ge(sem_pe, g + 1)
                bank = g % 2
                ve.tensor_copy(
                    out=g_t[:, 4 * g:4 * g + 4, :],
                    in_=ps[:, 4 * bank:4 * bank + 4, :],
                ).then_inc(sem_dve, 1)

        @blk.gpsimd
        def _(g):
            g_scatter = g_t[:].rearrange("p (q m) e -> p q (m e)", q=2)
            g.wait_ge(sem_sidx, 16)
            for q in range(2):
                g.wait_ge(sem_dve, (NG // 2) * (q + 1))
                for r in range(T):
                    g.indirect_dma_start(
                        out=out[:],
                        out_offset=bass.IndirectOffsetOnAxis(
                            ap=sidx_t[:, r, q:q + 1], axis=0),
                        in_=g_scatter[:, q, :],
                        in_offset=None,
                        bounds_check=NI - 1,
                        oob_is_err=False,
                    ).then_inc(sem_out, 16)
            g.wait_ge(sem_out, 16 * 2 * T)

    nc.compile()
    return nc


# --------------------------------------------------------------- v10 ----
# v9 kernel unchanged; host prep balances the token -> partition-slot
# assignment.  SDMA descriptor->engine assignment follows the partition
# swizzle (engine 2u <- partitions {4u..4u+3, 32+4u..35+4u}; odd engines
# the same pattern on partitions 64..127 -- verified against measured
# per-engine descriptor counts), so per-engine drain time is proportional
# to the occurrence-count sum of its partitions' tokens.  Greedy LPT over
# the 16 engine classes (+ big/small pairing within a class) equalizes
# per-engine load, compressing the drain ramp-down.
def _swizzle_class(p):
    if p < 64:
        return 2 * ((p % 32) // 4)
    return 2 * (((p - 64) % 32) // 4) + 1


_CLASS_PARTS = None


def _class_parts():
    global _CLASS_PARTS
    if _CLASS_PARTS is None:
        parts = [[] for _ in range(16)]
        for p in range(128):
            parts[_swizzle_class(p)].append(p)
        _CLASS_PARTS = parts
    return _CLASS_PARTS


def _balanced_slot_tok(counts):
    """slot_tok[q, p] = token for scatter slot (partition p, half q)."""
    order = np.argsort(-counts, kind="stable")
    class_sum = np.zeros(16, dtype=np.int64)
    class_toks = [[] for _ in range(16)]
    for tok in order:
        k = min((k for k in range(16) if len(class_toks[k]) < 16),
                key=lambda k: class_sum[k])
        class_toks[k].append(tok)
        class_sum[k] += counts[tok]
    slot_tok = np.empty((2, 128), dtype=np.int64)
    for k, ps in enumerate(_class_parts()):
        toks = class_toks[k]  # 16 tokens, descending count
        for i, p in enumerate(ps):
            slot_tok[0, p] = toks[i]
            slot_tok[1, p] = toks[15 - i]
    return slot_tok


def _prep_v10(nbr16_b, T):
    flat = nbr16_b.reshape(-1)
    counts = np.bincount(flat, minlength=At)
    slot_tok = _balanced_slot_tok(counts)

    nbr_r = nbr16_b[slot_tok].astype(np.int64)  # [q, j, m]
    pt = np.zeros((128, 2, Nbr, 2, 128), dtype=np.float16)
    q_ix, j_ix, m_ix = np.meshgrid(np.arange(2), np.arange(128),
                                   np.arange(Nbr), indexing="ij")
    vals = nbr_r[q_ix, j_ix, m_ix]
    pt[vals % 128, q_ix, m_ix, vals // 128, j_ix] = np.float16(1.0)
    ptd = np.ascontiguousarray(pt.reshape(128, NT * 2, 128)).astype(
        mybir.dt.np(mybir.dt.float8e4))

    tbl = _occurrence_tbl(flat, T)
    sidx = np.empty((128, T, 2), dtype=np.int32)
    for q in range(2):
        sidx[:, :, q] = tbl[slot_tok[q], :]
    return {"ptd": ptd, "sidx": sidx}


# ---------------------------------------------------------------- v2 ----
def _prep_v2(nbr16_b, T):
    tbl = _occurrence_tbl(nbr16_b.reshape(-1), T)
    sidx = np.empty((128, T, 2), dtype=np.int32)
    for q in range(2):
        sidx[:, :, q] = tbl[q * 128:(q + 1) * 128, :]
    return {"gidx": _prep_gidx(nbr16_b), "sidx": sidx}


def _build_nc_v2(T):
    nc = bacc.Bacc("TRN2", target_bir_lowering=False, debug=False)
    emb = nc.dram_tensor("emb", [At, F], mybir.dt.float32, kind="ExternalInput")
    gidx = nc.dram_tensor("gidx", [128, NI // 16], mybir.dt.int16, kind="ExternalInput")
    sidx = nc.dram_tensor("sidx", [128, T, 2], mybir.dt.int32, kind="ExternalInput")
    out = nc.dram_tensor("out", [NI, ROW], mybir.dt.float32, kind="ExternalOutput")

    with tile.TileContext(nc) as tc:
        with tc.tile_pool(name="pool0", bufs=1) as pool0:
            idx_t = pool0.tile([128, NI // 16], mybir.dt.int16)
            nc.sync.dma_start(idx_t[:], gidx[:])
            sidx_t = pool0.tile([128, T, 2], mybir.dt.int32)
            nc.sync.dma_start(sidx_t[:], sidx[:])

            g_t = pool0.tile([128, NI // 128, F], mybir.dt.float32)
            nc.gpsimd.dma_gather(g_t[:], emb[:], idx_t[:], NI, NI, F,
                                 single_packet=False)

            g_scatter = g_t[:].rearrange("p (q m) e -> p q (m e)", q=2)
            for r in range(T):
                for q in range(2):
                    nc.gpsimd.indirect_dma_start(
                        out=out[:],
                        out_offset=bass.IndirectOffsetOnAxis(
                            ap=sidx_t[:, r, q:q + 1], axis=0),
                        in_=g_scatter[:, q, :],
                        in_offset=None,
                        bounds_check=NI - 1,
                        oob_is_err=False,
                    )
    nc.compile()
    return nc


# ------------------------------------------------------------- driver ----
def _run(nc, in_maps, **kwargs):
    return run_bass_kernel_spmd(nc, in_maps, core_ids=list(range(B)), **kwargs)


def kernel(node_embedding: np.ndarray, nbr_idx: np.ndarray, _collect=None) -> np.ndarray:
    node_embedding = np.ascontiguousarray(node_embedding, dtype=np.float32)
    nbr16 = nbr_idx.astype(np.int16)  # values in [0, 256)

    T = int(max(np.bincount(nbr16[b].reshape(-1), minlength=At).max()
                for b in range(B)))
    key = (VERSION, T)
    builders = {"v2": _build_nc_v2, "v3": _build_nc_v3, "v4": _build_nc_v4,
                "v5": _build_nc_v5, "v6": _build_nc_v6, "v7": _build_nc_v7,
                "v8": _build_nc_v8, "v9": _build_nc_v9, "v10": _build_nc_v9}
    preps = {"v2": _prep_v2, "v3": _prep_v3, "v4": _prep_v4, "v5": _prep_v5,
             "v6": _prep_v6, "v7": _prep_v7, "v8": _prep_v8, "v9": _prep_v9,
             "v10": _prep_v10}
    if key not in _CACHED:
        _CACHED[key] = builders[VERSION](T)
    nc = _CACHED[key]
    if VERSION in ("v5", "v6", "v7", "v8", "v9", "v10"):
        in_maps = [{"emb16": np.ascontiguousarray(
                        node_embedding[b].reshape(2, 128, F)
                        .transpose(1, 0, 2)).astype(np.float16),
                    **preps[VERSION](nbr16[b], T)}
                   for b in range(B)]
    else:
        in_maps = [{"emb": node_embedding[b], **preps[VERSION](nbr16[b], T)}
                   for b in range(B)]

    res = _run(nc, in_maps)
    if _collect is not None:
        _collect.append(res)
    outs = [res.results[b]["out"].reshape(At, Nbr, Nbr, F) for b in range(B)]
    return np.stack(outs, axis=0)


# revision 24
# speedup vs baseline: 1.1315x; 1.1315x over previous
"""Trainium2 Bass kernel for nn_GetNodeK (gnn_message_passing).

out[b,i,n,m,:] = node_embedding[b, nbr_idx[b, nbr_idx[b,i,n], m], :]

Sharding: data-parallel over B (8 batches -> 8 cores, one batch per core).

Let nbr_flat = nbr_idx[b].reshape(6144) (values < 256) and define the
one-hop table G[j] = concat_m emb[nbr[j,m]] (256 rows x 12 KB = 3.1 MB).
Then out[b, k=(i*24+n)] = G[nbr_flat[k]] -- the 2-hop gather factors into
two index-driven stages that both use the raw nbr values (no chained
index arithmetic anywhere).

v9 (default, ~225 us vs 496 us baseline): raw bass (no TileContext).
G is built by TensorE permutation matmuls from host-uploaded exact fp16
one-hot matrices (PSUM f32, exactly one nonzero term per output -> the
only error is fp16 rounding of emb, rel err ~5e-3 << 2e-2); DVE copies
PSUM->SBUF.  The Pool engine runs ONLY the 2T [128,1]-offset indirect
scatter rounds back-to-back (manual semaphores, no conservative WAW deps
between rounds), so the 16 SDMA engines stream the 75.5 MB output write
at their ~390 GB/s aggregate line rate with no serialization.

Earlier iterations kept as fallback: v2 (tile, serialized rounds,
~496 us), v4 (raw bass + dma_gather stage 1, ~276 us), v6/v7 (v5 with
chunked pt loads / 6 KB row-piece scatters -- measured no better).
"""
import numpy as np

from concourse import bass, bacc, mybir
import concourse.tile as tile
from concourse.bass_utils import run_bass_kernel_spmd

B, At, Nbr, F = 8, 256, 24, 128
NI = At * Nbr        # 6144 indices per batch
ROW = Nbr * F        # 3072 f32 = 12 KB per stage-2 row
HALF = NI // 2       # 3072 gather indices per token-half
OOB = 8192           # idx sentinel > NI-1 -> skipped by bounds_check

VERSION = "v11"
_CACHED = {}


_T_PERM = None


def _v1_perm():
    """idx1[t] = nbr[(t//128//24)*128 + t%128, (t//128)%24] as flat index."""
    global _T_PERM
    if _T_PERM is None:
        t = np.arange(NI)
        s, p = t // 128, t % 128
        j, m = (s // Nbr) * 128 + p, s % Nbr
        _T_PERM = j * Nbr + m
    return _T_PERM


def _prep_gidx(nbr16_b):
    idx1 = nbr16_b.reshape(-1)[_v1_perm()]
    return np.tile(idx1.reshape(NI // 16, 16).T, (8, 1))


def _occurrence_tbl(flat, T):
    """tbl[j, t] = flat position of the t-th occurrence of token j."""
    counts = np.bincount(flat, minlength=At)
    order = np.argsort(flat, kind="stable")
    tbl = np.full((At, T), OOB, dtype=np.int32)
    pos = 0
    for j in range(At):
        c = counts[j]
        tbl[j, :c] = order[pos:pos + c]
        pos += c
    return tbl


# ---------------------------------------------------------------- v3 ----
def _prep_v3(nbr16_b, T):
    tbl = _occurrence_tbl(nbr16_b.reshape(-1), T)
    sidx = np.ascontiguousarray(tbl.reshape(2, 128, T).transpose(1, 0, 2))
    return {"gidx": _prep_gidx(nbr16_b), "sidx": sidx}


def _build_nc_v3(T):
    nc = bacc.Bacc("TRN2", target_bir_lowering=False, debug=False)
    emb = nc.dram_tensor("emb", [At, F], mybir.dt.float32, kind="ExternalInput")
    gidx = nc.dram_tensor("gidx", [128, NI // 16], mybir.dt.int16, kind="ExternalInput")
    sidx = nc.dram_tensor("sidx", [128, 2, T], mybir.dt.int32, kind="ExternalInput")
    out = nc.dram_tensor("out", [NI, ROW], mybir.dt.float32, kind="ExternalOutput")

    with tile.TileContext(nc) as tc:
        with tc.tile_pool(name="pool0", bufs=1) as pool0:
            idx_t = pool0.tile([128, NI // 16], mybir.dt.int16)
            nc.sync.dma_start(idx_t[:], gidx[:])
            sidx_t = pool0.tile([128, 2, T], mybir.dt.int32)
            nc.sync.dma_start(sidx_t[:], sidx[:])

            g_t = pool0.tile([128, NI // 128, F], mybir.dt.float32)
            g_scatter = g_t[:].rearrange("p (q m) e -> p q (m e)", q=2)
            for q in range(2):
                nc.gpsimd.dma_gather(
                    g_t[:, q * Nbr:(q + 1) * Nbr, :], emb[:],
                    idx_t[:, q * (HALF // 16):(q + 1) * (HALF // 16)],
                    HALF, HALF, F, single_packet=False,
                )
                src = g_scatter[:, q, :].unsqueeze(1).to_broadcast([128, T, ROW])
                nc.gpsimd.indirect_dma_start(
                    out=out[:],
                    out_offset=bass.IndirectOffsetOnAxis(
                        ap=sidx_t[:, q, :], axis=0),
                    in_=src,
                    in_offset=None,
                    bounds_check=NI - 1,
                    oob_is_err=False,
                )
    nc.compile()
    return nc


# ---------------------------------------------------------------- v4 ----
# Raw bass (no TileContext): identical per-round [128,1]-offset scatters as
# v2, but without Tile's conservative WAW deps between rounds -- the Pool
# engine issues all 2T descriptor-generation ops back-to-back and the 16
# SDMA engines drain continuously.  Sync is manual: idx loads -> gather
# half -> that half's T scatter rounds; final wait on the scatter sem.
def _prep_v4(nbr16_b, T):
    return _prep_v2(nbr16_b, T)


def _build_nc_v4(T):
    nc = bacc.Bacc("TRN2", target_bir_lowering=False, debug=False)
    emb = nc.dram_tensor("emb", [At, F], mybir.dt.float32, kind="ExternalInput")
    gidx = nc.dram_tensor("gidx", [128, NI // 16], mybir.dt.int16, kind="ExternalInput")
    sidx = nc.dram_tensor("sidx", [128, T, 2], mybir.dt.int32, kind="ExternalInput")
    out = nc.dram_tensor("out", [NI, ROW], mybir.dt.float32, kind="ExternalOutput")

    idx_t = nc.alloc_sbuf_tensor("idx_t", [128, NI // 16], mybir.dt.int16)
    sidx_t = nc.alloc_sbuf_tensor("sidx_t", [128, T, 2], mybir.dt.int32)
    g_t = nc.alloc_sbuf_tensor("g_t", [128, NI // 128, F], mybir.dt.float32)

    sem_idx = nc.alloc_semaphore("sem_idx")
    sem_g = nc.alloc_semaphore("sem_g")
    sem_out = nc.alloc_semaphore("sem_out")

    with nc.Block() as blk:

        @blk.sync
        def _(sync):
            sync.dma_start(idx_t[:], gidx[:]).then_inc(sem_idx, 16)
            sync.dma_start(sidx_t[:], sidx[:]).then_inc(sem_idx, 16)

        @blk.gpsimd
        def _(g):
            g.wait_ge(sem_idx, 32)
            g_scatter = g_t[:].rearrange("p (q m) e -> p q (m e)", q=2)
            for q in range(2):
                g.dma_gather(
                    g_t[:, q * Nbr:(q + 1) * Nbr, :], emb[:],
                    idx_t[:, q * (HALF // 16):(q + 1) * (HALF // 16)],
                    HALF, HALF, F, single_packet=False,
                ).then_inc(sem_g, 16)
                g.wait_ge(sem_g, 16 * (q + 1))
                for r in range(T):
                    g.indirect_dma_start(
                        out=out[:],
                        out_offset=bass.IndirectOffsetOnAxis(
                            ap=sidx_t[:, r, q:q + 1], axis=0),
                        in_=g_scatter[:, q, :],
                        in_offset=None,
                        bounds_check=NI - 1,
                        oob_is_err=False,
                    ).then_inc(sem_out, 16)
            g.wait_ge(sem_out, 16 * 2 * T)

    nc.compile()
    return nc


# ---------------------------------------------------------------- v5 ----
# v4 + stage-1 gather moved off the GpSimd/DMA path entirely: G is built by
# TensorE permutation matmuls.  Host uploads exact fp16 one-hot matrices
# PT[(q*24+m)*2+h][i, j] = (nbr[q*128+j, m] == h*128+i); per (q,m) tile
# G[j, :] = PT_lo.T @ emb_lo + PT_hi.T @ emb_hi accumulates in PSUM (f32,
# exactly one nonzero term -> result is just emb rounded to fp16, rel err
# ~2^-11 << 2e-2 gate).  DVE copies PSUM->SBUF.  The Pool engine runs ONLY
# the 2T indirect-scatter rounds, and the 16 SDMA engines carry nothing but
# the 75.5 MB output write.
NT = 2 * Nbr         # 48 (q,m) tiles
NG = NT // 4         # 12 groups of 4 tiles (one PSUM bank each)


def _prep_v5(nbr16_b, T):
    nbr_r = nbr16_b.reshape(2, 128, Nbr).astype(np.int64)  # [q, j, m]
    pt = np.zeros((128, 2, Nbr, 2, 128), dtype=np.float16)  # [i, q, m, h, j]
    q_ix, j_ix, m_ix = np.meshgrid(np.arange(2), np.arange(128),
                                   np.arange(Nbr), indexing="ij")
    vals = nbr_r[q_ix, j_ix, m_ix]
    pt[vals % 128, q_ix, m_ix, vals // 128, j_ix] = np.float16(1.0)
    ptd = np.ascontiguousarray(pt.reshape(128, NT * 2, 128))

    tbl = _occurrence_tbl(nbr16_b.reshape(-1), T)
    sidx = np.empty((128, T, 2), dtype=np.int32)
    for q in range(2):
        sidx[:, :, q] = tbl[q * 128:(q + 1) * 128, :]
    return {"ptd": ptd, "sidx": sidx}


def _build_nc_v5(T):
    nc = bacc.Bacc("TRN2", target_bir_lowering=False, debug=False)
    emb16d = nc.dram_tensor("emb16", [128, 2, F], mybir.dt.float16, kind="ExternalInput")
    ptd = nc.dram_tensor("ptd", [128, NT * 2, 128], mybir.dt.float16, kind="ExternalInput")
    sidxd = nc.dram_tensor("sidx", [128, T, 2], mybir.dt.int32, kind="ExternalInput")
    out = nc.dram_tensor("out", [NI, ROW], mybir.dt.float32, kind="ExternalOutput")

    emb_t = nc.alloc_sbuf_tensor("emb_t", [128, 2, F], mybir.dt.float16)
    pt_t = nc.alloc_sbuf_tensor("pt_t", [128, NT * 2, 128], mybir.dt.float16)
    sidx_t = nc.alloc_sbuf_tensor("sidx_t", [128, T, 2], mybir.dt.int32)
    g_t = nc.alloc_sbuf_tensor("g_t", [128, NI // 128, F], mybir.dt.float32)
    ps = nc.alloc_psum_tensor("ps", [128, 8, 128], mybir.dt.float32)

    sem_in = nc.alloc_semaphore("sem_in")
    sem_in2 = nc.alloc_semaphore("sem_in2")
    sem_sidx = nc.alloc_semaphore("sem_sidx")
    sem_pe = nc.alloc_semaphore("sem_pe")
    sem_dve = nc.alloc_semaphore("sem_dve")
    sem_out = nc.alloc_semaphore("sem_out")

    with nc.Block() as blk:

        @blk.sync
        def _(sync):
            sync.dma_start(emb_t[:], emb16d[:]).then_inc(sem_in, 16)
            # pt halves separately so PE can start on half 0 sooner
            sync.dma_start(pt_t[:, :NT, :], ptd[:, :NT, :]).then_inc(sem_in, 16)
            sync.dma_start(pt_t[:, NT:, :], ptd[:, NT:, :]).then_inc(sem_in2, 16)
            sync.dma_start(sidx_t[:], sidxd[:]).then_inc(sem_sidx, 16)

        @blk.tensor
        def _(te):
            te.wait_ge(sem_in, 32)  # emb + pt half 0
            for g in range(NG):
                if g == NG // 2:
                    te.wait_ge(sem_in2, 16)  # pt half 1
                if g >= 2:
                    te.wait_ge(sem_dve, g - 1)  # bank g%2 reusable
                bank = g % 2
                for k in range(4):
                    s = 4 * g + k
                    te.matmul(out=ps[:, 4 * bank + k, :],
                              lhsT=pt_t[:, 2 * s, :], rhs=emb_t[:, 0, :],
                              start=True, stop=False)
                    mm = te.matmul(out=ps[:, 4 * bank + k, :],
                                   lhsT=pt_t[:, 2 * s + 1, :], rhs=emb_t[:, 1, :],
                                   start=False, stop=True)
                    if k == 3:
                        mm.then_inc(sem_pe, 1)

        @blk.vector
        def _(ve):
            for g in range(NG):
                ve.wait_ge(sem_pe, g + 1)
                bank = g % 2
                ve.tensor_copy(
                    out=g_t[:, 4 * g:4 * g + 4, :],
                    in_=ps[:, 4 * bank:4 * bank + 4, :],
                ).then_inc(sem_dve, 1)

        @blk.gpsimd
        def _(g):
            g_scatter = g_t[:].rearrange("p (q m) e -> p q (m e)", q=2)
            g.wait_ge(sem_sidx, 16)
            for q in range(2):
                g.wait_ge(sem_dve, (NG // 2) * (q + 1))
                for r in range(T):
                    g.indirect_dma_start(
                        out=out[:],
                        out_offset=bass.IndirectOffsetOnAxis(
                            ap=sidx_t[:, r, q:q + 1], axis=0),
                        in_=g_scatter[:, q, :],
                        in_offset=None,
                        bounds_check=NI - 1,
                        oob_is_err=False,
                    ).then_inc(sem_out, 16)
            g.wait_ge(sem_out, 16 * 2 * T)

    nc.compile()
    return nc


# ---------------------------------------------------------------- v6 ----
# v5 with the pt upload split into 4 chunks (own semaphores, FIFO HWDGE)
# so the PE can start building G as soon as the first 12 tiles land,
# pulling the first scatter round ~5us earlier.
def _prep_v6(nbr16_b, T):
    return _prep_v5(nbr16_b, T)


def _build_nc_v6(T):
    nc = bacc.Bacc("TRN2", target_bir_lowering=False, debug=False)
    emb16d = nc.dram_tensor("emb16", [128, 2, F], mybir.dt.float16, kind="ExternalInput")
    ptd = nc.dram_tensor("ptd", [128, NT * 2, 128], mybir.dt.float16, kind="ExternalInput")
    sidxd = nc.dram_tensor("sidx", [128, T, 2], mybir.dt.int32, kind="ExternalInput")
    out = nc.dram_tensor("out", [NI, ROW], mybir.dt.float32, kind="ExternalOutput")

    emb_t = nc.alloc_sbuf_tensor("emb_t", [128, 2, F], mybir.dt.float16)
    pt_t = nc.alloc_sbuf_tensor("pt_t", [128, NT * 2, 128], mybir.dt.float16)
    sidx_t = nc.alloc_sbuf_tensor("sidx_t", [128, T, 2], mybir.dt.int32)
    g_t = nc.alloc_sbuf_tensor("g_t", [128, NI // 128, F], mybir.dt.float32)
    ps = nc.alloc_psum_tensor("ps", [128, 8, 128], mybir.dt.float32)

    sem_emb = nc.alloc_semaphore("sem_emb")
    sem_pt = [nc.alloc_semaphore(f"sem_pt{c}") for c in range(4)]
    sem_sidx = nc.alloc_semaphore("sem_sidx")
    sem_pe = nc.alloc_semaphore("sem_pe")
    sem_dve = nc.alloc_semaphore("sem_dve")
    sem_out = nc.alloc_semaphore("sem_out")

    CH = NT * 2 // 4  # 24 pt tiles per chunk = 3 groups

    with nc.Block() as blk:

        @blk.sync
        def _(sync):
            sync.dma_start(emb_t[:], emb16d[:]).then_inc(sem_emb, 16)
            for c in range(4):
                sync.dma_start(pt_t[:, c * CH:(c + 1) * CH, :],
                               ptd[:, c * CH:(c + 1) * CH, :]).then_inc(sem_pt[c], 16)
            sync.dma_start(sidx_t[:], sidxd[:]).then_inc(sem_sidx, 16)

        @blk.tensor
        def _(te):
            te.wait_ge(sem_emb, 16)
            for g in range(NG):
                if g % 3 == 0:
                    te.wait_ge(sem_pt[g // 3], 16)
                if g >= 2:
                    te.wait_ge(sem_dve, g - 1)
                bank = g % 2
                for k in range(4):
                    s = 4 * g + k
                    te.matmul(out=ps[:, 4 * bank + k, :],
                              lhsT=pt_t[:, 2 * s, :], rhs=emb_t[:, 0, :],
                              start=True, stop=False)
                    mm = te.matmul(out=ps[:, 4 * bank + k, :],
                                   lhsT=pt_t[:, 2 * s + 1, :], rhs=emb_t[:, 1, :],
                                   start=False, stop=True)
                    if k == 3:
                        mm.then_inc(sem_pe, 1)

        @blk.vector
        def _(ve):
            for g in range(NG):
                ve.wait_ge(sem_pe, g + 1)
                bank = g % 2
                ve.tensor_copy(
                    out=g_t[:, 4 * g:4 * g + 4, :],
                    in_=ps[:, 4 * bank:4 * bank + 4, :],
                ).then_inc(sem_dve, 1)

        @blk.gpsimd
        def _(g):
            g_scatter = g_t[:].rearrange("p (q m) e -> p q (m e)", q=2)
            g.wait_ge(sem_sidx, 16)
            for q in range(2):
                g.wait_ge(sem_dve, (NG // 2) * (q + 1))
                for r in range(T):
                    g.indirect_dma_start(
                        out=out[:],
                        out_offset=bass.IndirectOffsetOnAxis(
                            ap=sidx_t[:, r, q:q + 1], axis=0),
                        in_=g_scatter[:, q, :],
                        in_offset=None,
                        bounds_check=NI - 1,
                        oob_is_err=False,
                    ).then_inc(sem_out, 16)
            g.wait_ge(sem_out, 16 * 2 * T)

    nc.compile()
    return nc


# ---------------------------------------------------------------- v7 ----
# v6 + earlier drain start: half 0 is scattered as two 6 KB row-pieces
# against a [2*NI, ROW/2] view of out (row 2k+h = columns [h*1536,(h+1)*1536)
# of out row k -- same memory, offset 0, so no element_offset needed).
# Piece 0 only needs m-slots 0..11 (PE groups 0-2), pulling the first
# scatter ~5us earlier; half 1 stays full-row 12 KB.
def _prep_v7(nbr16_b, T):
    d = _prep_v5(nbr16_b, T)
    tbl = _occurrence_tbl(nbr16_b.reshape(-1), T)  # [At, T], OOB-padded
    h0 = tbl[:128, :]                              # tokens 0..127
    sidx2 = np.empty((128, T, 2), dtype=np.int32)  # [p, r, h] -> 2*row+h
    for h in range(2):
        sidx2[:, :, h] = 2 * h0 + h
    d["sidx2"] = sidx2
    return d


def _build_nc_v7(T):
    nc = bacc.Bacc("TRN2", target_bir_lowering=False, debug=False)
    emb16d = nc.dram_tensor("emb16", [128, 2, F], mybir.dt.float16, kind="ExternalInput")
    ptd = nc.dram_tensor("ptd", [128, NT * 2, 128], mybir.dt.float16, kind="ExternalInput")
    sidxd = nc.dram_tensor("sidx", [128, T, 2], mybir.dt.int32, kind="ExternalInput")
    sidx2d = nc.dram_tensor("sidx2", [128, T, 2], mybir.dt.int32, kind="ExternalInput")
    out = nc.dram_tensor("out", [NI, ROW], mybir.dt.float32, kind="ExternalOutput")

    emb_t = nc.alloc_sbuf_tensor("emb_t", [128, 2, F], mybir.dt.float16)
    pt_t = nc.alloc_sbuf_tensor("pt_t", [128, NT * 2, 128], mybir.dt.float16)
    sidx_t = nc.alloc_sbuf_tensor("sidx_t", [128, T, 2], mybir.dt.int32)
    sidx2_t = nc.alloc_sbuf_tensor("sidx2_t", [128, T, 2], mybir.dt.int32)
    g_t = nc.alloc_sbuf_tensor("g_t", [128, NI // 128, F], mybir.dt.float32)
    ps = nc.alloc_psum_tensor("ps", [128, 8, 128], mybir.dt.float32)

    sem_emb = nc.alloc_semaphore("sem_emb")
    sem_pt = [nc.alloc_semaphore(f"sem_pt{c}") for c in range(4)]
    sem_sidx = nc.alloc_semaphore("sem_sidx")
    sem_pe = nc.alloc_semaphore("sem_pe")
    sem_dve = nc.alloc_semaphore("sem_dve")
    sem_out = nc.alloc_semaphore("sem_out")

    CH = NT * 2 // 4

    with nc.Block() as blk:

        @blk.sync
        def _(sync):
            sync.dma_start(emb_t[:], emb16d[:]).then_inc(sem_emb, 16)
            for c in range(4):
                sync.dma_start(pt_t[:, c * CH:(c + 1) * CH, :],
                               ptd[:, c * CH:(c + 1) * CH, :]).then_inc(sem_pt[c], 16)
            sync.dma_start(sidx_t[:], sidxd[:]).then_inc(sem_sidx, 16)
            sync.dma_start(sidx2_t[:], sidx2d[:]).then_inc(sem_sidx, 16)

        @blk.tensor
        def _(te):
            te.wait_ge(sem_emb, 16)
            for g in range(NG):
                if g % 3 == 0:
                    te.wait_ge(sem_pt[g // 3], 16)
                if g >= 2:
                    te.wait_ge(sem_dve, g - 1)
                bank = g % 2
                for k in range(4):
                    s = 4 * g + k
                    te.matmul(out=ps[:, 4 * bank + k, :],
                              lhsT=pt_t[:, 2 * s, :], rhs=emb_t[:, 0, :],
                              start=True, stop=False)
                    mm = te.matmul(out=ps[:, 4 * bank + k, :],
                                   lhsT=pt_t[:, 2 * s + 1, :], rhs=emb_t[:, 1, :],
                                   start=False, stop=True)
                    if k == 3:
                        mm.then_inc(sem_pe, 1)

        @blk.vector
        def _(ve):
            for g in range(NG):
                ve.wait_ge(sem_pe, g + 1)
                bank = g % 2
                ve.tensor_copy(
                    out=g_t[:, 4 * g:4 * g + 4, :],
                    in_=ps[:, 4 * bank:4 * bank + 4, :],
                ).then_inc(sem_dve, 1)

        @blk.gpsimd
        def _(g):
            out2 = out[:].rearrange("k (h e) -> (k h) e", h=2)  # [2*NI, 1536]
            g_scatter = g_t[:].rearrange("p (q m) e -> p q (m e)", q=2)
            g_half = g_t[:].rearrange("p (x y) e -> p x (y e)", x=4)  # 6KB quarters
            g.wait_ge(sem_sidx, 32)
            nrounds = 0
            # half 0 as two 6KB pieces (piece h needs PE groups 0-2 / 3-5)
            for h in range(2):
                g.wait_ge(sem_dve, 3 * (h + 1))
                for r in range(T):
                    g.indirect_dma_start(
                        out=out2,
                        out_offset=bass.IndirectOffsetOnAxis(
                            ap=sidx2_t[:, r, h:h + 1], axis=0),
                        in_=g_half[:, h, :],
                        in_offset=None,
                        bounds_check=2 * NI - 1,
                        oob_is_err=False,
                    ).then_inc(sem_out, 16)
                    nrounds += 1
            # half 1 full 12KB rows
            g.wait_ge(sem_dve, NG)
            for r in range(T):
                g.indirect_dma_start(
                    out=out[:],
                    out_offset=bass.IndirectOffsetOnAxis(
                        ap=sidx_t[:, r, 1:2], axis=0),
                    in_=g_scatter[:, 1, :],
                    in_offset=None,
                    bounds_check=NI - 1,
                    oob_is_err=False,
                ).then_inc(sem_out, 16)
                nrounds += 1
            g.wait_ge(sem_out, 16 * nrounds)

    nc.compile()
    return nc


# ---------------------------------------------------------------- v8 ----
# v5 with: (a) PT one-hots in fp8e4 (0/1 exact; halves the 3MB upload that
# gates the PE start), (b) pt half 0 loaded before emb, (c) the block exit
# skips GpSimd's dge_drain (no_gpsimd_drain=True) -- the explicit sem_out
# wait already guarantees every output byte landed.
def _prep_v8(nbr16_b, T):
    d = _prep_v5(nbr16_b, T)
    d["ptd"] = d["ptd"].astype(mybir.dt.np(mybir.dt.float8e4))
    return d


def _build_nc_v8(T):
    nc = bacc.Bacc("TRN2", target_bir_lowering=False, debug=False)
    emb16d = nc.dram_tensor("emb16", [128, 2, F], mybir.dt.float16, kind="ExternalInput")
    ptd = nc.dram_tensor("ptd", [128, NT * 2, 128], mybir.dt.float8e4, kind="ExternalInput")
    sidxd = nc.dram_tensor("sidx", [128, T, 2], mybir.dt.int32, kind="ExternalInput")
    out = nc.dram_tensor("out", [NI, ROW], mybir.dt.float32, kind="ExternalOutput")

    emb_t = nc.alloc_sbuf_tensor("emb_t", [128, 2, F], mybir.dt.float16)
    pt_t = nc.alloc_sbuf_tensor("pt_t", [128, NT * 2, 128], mybir.dt.float8e4)
    sidx_t = nc.alloc_sbuf_tensor("sidx_t", [128, T, 2], mybir.dt.int32)
    g_t = nc.alloc_sbuf_tensor("g_t", [128, NI // 128, F], mybir.dt.float32)
    ps = nc.alloc_psum_tensor("ps", [128, 8, 128], mybir.dt.float32)

    sem_emb = nc.alloc_semaphore("sem_emb")
    sem_pt0 = nc.alloc_semaphore("sem_pt0")
    sem_pt1 = nc.alloc_semaphore("sem_pt1")
    sem_sidx = nc.alloc_semaphore("sem_sidx")
    sem_pe = nc.alloc_semaphore("sem_pe")
    sem_dve = nc.alloc_semaphore("sem_dve")
    sem_out = nc.alloc_semaphore("sem_out")

    with nc.Block(no_gpsimd_drain=True) as blk:

        @blk.sync
        def _(sync):
            sync.dma_start(pt_t[:, :NT, :], ptd[:, :NT, :]).then_inc(sem_pt0, 16)
            sync.dma_start(emb_t[:], emb16d[:]).then_inc(sem_emb, 16)
            sync.dma_start(pt_t[:, NT:, :], ptd[:, NT:, :]).then_inc(sem_pt1, 16)
            sync.dma_start(sidx_t[:], sidxd[:]).then_inc(sem_sidx, 16)

        @blk.tensor
        def _(te):
            te.wait_ge(sem_pt0, 16)
            te.wait_ge(sem_emb, 16)
            for g in range(NG):
                if g == NG // 2:
                    te.wait_ge(sem_pt1, 16)
                if g >= 2:
                    te.wait_ge(sem_dve, g - 1)
                bank = g % 2
                for k in range(4):
                    s = 4 * g + k
                    te.matmul(out=ps[:, 4 * bank + k, :],
                              lhsT=pt_t[:, 2 * s, :], rhs=emb_t[:, 0, :],
                              start=True, stop=False)
                    mm = te.matmul(out=ps[:, 4 * bank + k, :],
                                   lhsT=pt_t[:, 2 * s + 1, :], rhs=emb_t[:, 1, :],
                                   start=False, stop=True)
                    if k == 3:
                        mm.then_inc(sem_pe, 1)

        @blk.vector
        def _(ve):
            for g in range(NG):
                ve.wait_ge(sem_pe, g + 1)
                bank = g % 2
                ve.tensor_copy(
                    out=g_t[:, 4 * g:4 * g + 4, :],
                    in_=ps[:, 4 * bank:4 * bank + 4, :],
                ).then_inc(sem_dve, 1)

        @blk.gpsimd
        def _(g):
            g_scatter = g_t[:].rearrange("p (q m) e -> p q (m e)", q=2)
            g.wait_ge(sem_sidx, 16)
            for q in range(2):
                g.wait_ge(sem_dve, (NG // 2) * (q + 1))
                for r in range(T):
                    g.indirect_dma_start(
                        out=out[:],
                        out_offset=bass.IndirectOffsetOnAxis(
                            ap=sidx_t[:, r, q:q + 1], axis=0),
                        in_=g_scatter[:, q, :],
                        in_offset=None,
                        bounds_check=NI - 1,
                        oob_is_err=False,
                    ).then_inc(sem_out, 16)
            g.wait_ge(sem_out, 16 * 2 * T)

    nc.compile()
    return nc


# ---------------------------------------------------------------- v9 ----
# v8 with the pt upload split into 6 chunks of 16 tiles (2 PE groups each)
# so matmuls stream right behind the DMA instead of waiting for the full
# 0.75 MB half.
def _prep_v9(nbr16_b, T):
    return _prep_v8(nbr16_b, T)


def _build_nc_v9(T):
    nc = bacc.Bacc("TRN2", target_bir_lowering=False, debug=False)
    emb16d = nc.dram_tensor("emb16", [128, 2, F], mybir.dt.float16, kind="ExternalInput")
    ptd = nc.dram_tensor("ptd", [128, NT * 2, 128], mybir.dt.float8e4, kind="ExternalInput")
    sidxd = nc.dram_tensor("sidx", [128, T, 2], mybir.dt.int32, kind="ExternalInput")
    out = nc.dram_tensor("out", [NI, ROW], mybir.dt.float32, kind="ExternalOutput")

    emb_t = nc.alloc_sbuf_tensor("emb_t", [128, 2, F], mybir.dt.float16)
    pt_t = nc.alloc_sbuf_tensor("pt_t", [128, NT * 2, 128], mybir.dt.float8e4)
    sidx_t = nc.alloc_sbuf_tensor("sidx_t", [128, T, 2], mybir.dt.int32)
    g_t = nc.alloc_sbuf_tensor("g_t", [128, NI // 128, F], mybir.dt.float32)
    ps = nc.alloc_psum_tensor("ps", [128, 8, 128], mybir.dt.float32)

    sem_emb = nc.alloc_semaphore("sem_emb")
    sem_pt = [nc.alloc_semaphore(f"sem_pt{c}") for c in range(6)]
    sem_sidx = nc.alloc_semaphore("sem_sidx")
    sem_pe = nc.alloc_semaphore("sem_pe")
    sem_dve = nc.alloc_semaphore("sem_dve")
    sem_out = nc.alloc_semaphore("sem_out")

    CH = NT * 2 // 6  # 16 pt tiles per chunk = 2 PE groups

    with nc.Block(no_gpsimd_drain=True) as blk:

        @blk.sync
        def _(sync):
            sync.dma_start(emb_t[:], emb16d[:]).then_inc(sem_emb, 16)
            for c in range(6):
                sync.dma_start(pt_t[:, c * CH:(c + 1) * CH, :],
                               ptd[:, c * CH:(c + 1) * CH, :]).then_inc(sem_pt[c], 16)
            sync.dma_start(sidx_t[:], sidxd[:]).then_inc(sem_sidx, 16)

        @blk.tensor
        def _(te):
            te.wait_ge(sem_emb, 16)
            for g in range(NG):
                if g % 2 == 0:
                    te.wait_ge(sem_pt[g // 2], 16)
                if g >= 2:
                    te.wait_ge(sem_dve, g - 1)
                bank = g % 2
                for k in range(4):
                    s = 4 * g + k
                    te.matmul(out=ps[:, 4 * bank + k, :],
                              lhsT=pt_t[:, 2 * s, :], rhs=emb_t[:, 0, :],
                              start=True, stop=False)
                    mm = te.matmul(out=ps[:, 4 * bank + k, :],
                                   lhsT=pt_t[:, 2 * s + 1, :], rhs=emb_t[:, 1, :],
                                   start=False, stop=True)
                    if k == 3:
                        mm.then_inc(sem_pe, 1)

        @blk.vector
        def _(ve):
            for g in range(NG):
                ve.wait_ge(sem_pe, g + 1)
                bank = g % 2
                ve.tensor_copy(
                    out=g_t[:, 4 * g:4 * g + 4, :],
                    in_=ps[:, 4 * bank:4 * bank + 4, :],
                ).then_inc(sem_dve, 1)

        @blk.gpsimd
        def _(g):
            g_scatter = g_t[:].rearrange("p (q m) e -> p q (m e)", q=2)
            g.wait_ge(sem_sidx, 16)
            for q in range(2):
                g.wait_ge(sem_dve, (NG // 2) * (q + 1))
                for r in range(T):
                    g.indirect_dma_start(
                        out=out[:],
                        out_offset=bass.IndirectOffsetOnAxis(
                            ap=sidx_t[:, r, q:q + 1], axis=0),
                        in_=g_scatter[:, q, :],
                        in_offset=None,
                        bounds_check=NI - 1,
                        oob_is_err=False,
                    ).then_inc(sem_out, 16)
            g.wait_ge(sem_out, 16 * 2 * T)

    nc.compile()
    return nc


# --------------------------------------------------------------- v10 ----
# v9 kernel unchanged; host prep balances the token -> partition-slot
# assignment.  SDMA descriptor->engine assignment follows the partition
# swizzle (engine 2u <- partitions {4u..4u+3, 32+4u..35+4u}; odd engines
# the same pattern on partitions 64..127 -- verified against measured
# per-engine descriptor counts), so per-engine drain time is proportional
# to the occurrence-count sum of its partitions' tokens.  Greedy LPT over
# the 16 engine classes (+ big/small pairing within a class) equalizes
# per-engine load, compressing the drain ramp-down.
def _swizzle_class(p):
    if p < 64:
        return 2 * ((p % 32) // 4)
    return 2 * (((p - 64) % 32) // 4) + 1


_CLASS_PARTS = None


def _class_parts():
    global _CLASS_PARTS
    if _CLASS_PARTS is None:
        parts = [[] for _ in range(16)]
        for p in range(128):
            parts[_swizzle_class(p)].append(p)
        _CLASS_PARTS = parts
    return _CLASS_PARTS


def _balanced_slot_tok(counts):
    """slot_tok[q, p] = token for scatter slot (partition p, half q)."""
    order = np.argsort(-counts, kind="stable")
    class_sum = np.zeros(16, dtype=np.int64)
    class_toks = [[] for _ in range(16)]
    for tok in order:
        k = min((k for k in range(16) if len(class_toks[k]) < 16),
                key=lambda k: class_sum[k])
        class_toks[k].append(tok)
        class_sum[k] += counts[tok]
    slot_tok = np.empty((2, 128), dtype=np.int64)
    for k, ps in enumerate(_class_parts()):
        toks = class_toks[k]  # 16 tokens, descending count
        for i, p in enumerate(ps):
            slot_tok[0, p] = toks[i]
            slot_tok[1, p] = toks[15 - i]
    return slot_tok


def _prep_v10(nbr16_b, T):
    flat = nbr16_b.reshape(-1)
    counts = np.bincount(flat, minlength=At)
    slot_tok = _balanced_slot_tok(counts)

    nbr_r = nbr16_b[slot_tok].astype(np.int64)  # [q, j, m]
    pt = np.zeros((128, 2, Nbr, 2, 128), dtype=np.float16)
    q_ix, j_ix, m_ix = np.meshgrid(np.arange(2), np.arange(128),
                                   np.arange(Nbr), indexing="ij")
    vals = nbr_r[q_ix, j_ix, m_ix]
    pt[vals % 128, q_ix, m_ix, vals // 128, j_ix] = np.float16(1.0)
    ptd = np.ascontiguousarray(pt.reshape(128, NT * 2, 128)).astype(
        mybir.dt.np(mybir.dt.float8e4))

    tbl = _occurrence_tbl(flat, T)
    sidx = np.empty((128, T, 2), dtype=np.int32)
    for q in range(2):
        sidx[:, :, q] = tbl[slot_tok[q], :]
    return {"ptd": ptd, "sidx": sidx}


# --------------------------------------------------------------- v11 ----
# v10 with WEIGHTED balance: SDMA engine 15 services SWDGE descriptors
# ~19% slower than the rest (known 7/15 erratum; measured uniform
# 560ns vs 467ns per 12KB descriptor), so equal byte-balance makes it the
# critical path.  Weight its class by 0.83 so all engines finish together.
_ENGINE_SPEED = np.array([1.0] * 15 + [0.83])


def _weighted_slot_tok(counts):
    order = np.argsort(-counts, kind="stable")
    class_sum = np.zeros(16, dtype=np.float64)
    class_toks = [[] for _ in range(16)]
    for tok in order:
        k = min((k for k in range(16) if len(class_toks[k]) < 16),
                key=lambda k: (class_sum[k] + counts[tok]) / _ENGINE_SPEED[k])
        class_toks[k].append(tok)
        class_sum[k] += counts[tok]
    slot_tok = np.empty((2, 128), dtype=np.int64)
    for k, ps in enumerate(_class_parts()):
        toks = class_toks[k]
        for i, p in enumerate(ps):
            slot_tok[0, p] = toks[i]
            slot_tok[1, p] = toks[15 - i]
    return slot_tok


def _prep_v11(nbr16_b, T):
    flat = nbr16_b.reshape(-1)
    counts = np.bincount(flat, minlength=At)
    slot_tok = _weighted_slot_tok(counts)

    nbr_r = nbr16_b[slot_tok].astype(np.int64)
    pt = np.zeros((128, 2, Nbr, 2, 128), dtype=np.float16)
    q_ix, j_ix, m_ix = np.meshgrid(np.arange(2), np.arange(128),
                                   np.arange(Nbr), indexing="ij")
    vals = nbr_r[q_ix, j_ix, m_ix]
    pt[vals % 128, q_ix, m_ix, vals // 128, j_ix] = np.float16(1.0)
    ptd = np.ascontiguousarray(pt.reshape(128, NT * 2, 128)).astype(
        mybir.dt.np(mybir.dt.float8e4))

    tbl = _occurrence_tbl(flat, T)
    sidx = np.empty((128, T, 2), dtype=np.int32)
    for q in range(2):
        sidx[:, :, q] = tbl[slot_tok[q], :]
    return {"ptd": ptd, "sidx": sidx}


# ---------------------------------------------------------------- v2 ----
def _prep_v2(nbr16_b, T):
    tbl = _occurrence_tbl(nbr16_b.reshape(-1), T)
    sidx = np.empty((128, T, 2), dtype=np.int32)
    for q in range(2):
        sidx[:, :, q] = tbl[q * 128:(q + 1) * 128, :]
    return {"gidx": _prep_gidx(nbr16_b), "sidx": sidx}


def _build_nc_v2(T):
    nc = bacc.Bacc("TRN2", target_bir_lowering=False, debug=False)
    emb = nc.dram_tensor("emb", [At, F], mybir.dt.float32, kind="ExternalInput")
    gidx = nc.dram_tensor("gidx", [128, NI // 16], mybir.dt.int16, kind="ExternalInput")
    sidx = nc.dram_tensor("sidx", [128, T, 2], mybir.dt.int32, kind="ExternalInput")
    out = nc.dram_tensor("out", [NI, ROW], mybir.dt.float32, kind="ExternalOutput")

    with tile.TileContext(nc) as tc:
        with tc.tile_pool(name="pool0", bufs=1) as pool0:
            idx_t = pool0.tile([128, NI // 16], mybir.dt.int16)
            nc.sync.dma_start(idx_t[:], gidx[:])
            sidx_t = pool0.tile([128, T, 2], mybir.dt.int32)
            nc.sync.dma_start(sidx_t[:], sidx[:])

            g_t = pool0.tile([128, NI // 128, F], mybir.dt.float32)
            nc.gpsimd.dma_gather(g_t[:], emb[:], idx_t[:], NI, NI, F,
                                 single_packet=False)

            g_scatter = g_t[:].rearrange("p (q m) e -> p q (m e)", q=2)
            for r in range(T):
                for q in range(2):
                    nc.gpsimd.indirect_dma_start(
                        out=out[:],
                        out_offset=bass.IndirectOffsetOnAxis(
                            ap=sidx_t[:, r, q:q + 1], axis=0),
                        in_=g_scatter[:, q, :],
                        in_offset=None,
                        bounds_check=NI - 1,
                        oob_is_err=False,
                    )
    nc.compile()
    return nc


# ------------------------------------------------------------- driver ----
def _run(nc, in_maps, **kwargs):
    return run_bass_kernel_spmd(nc, in_maps, core_ids=list(range(B)), **kwargs)


def kernel(node_embedding: np.ndarray, nbr_idx: np.ndarray, _collect=None) -> np.ndarray:
    node_embedding = np.ascontiguousarray(node_embedding, dtype=np.float32)
    nbr16 = nbr_idx.astype(np.int16)  # values in [0, 256)

    T = int(max(np.bincount(nbr16[b].reshape(-1), minlength=At).max()
                for b in range(B)))
    key = (VERSION, T)
    builders = {"v2": _build_nc_v2, "v3": _build_nc_v3, "v4": _build_nc_v4,
                "v5": _build_nc_v5, "v6": _build_nc_v6, "v7": _build_nc_v7,
                "v8": _build_nc_v8, "v9": _build_nc_v9, "v10": _build_nc_v9,
                "v11": _build_nc_v9}
    preps = {"v2": _prep_v2, "v3": _prep_v3, "v4": _prep_v4, "v5": _prep_v5,
             "v6": _prep_v6, "v7": _prep_v7, "v8": _prep_v8, "v9": _prep_v9,
             "v10": _prep_v10, "v11": _prep_v11}
    if key not in _CACHED:
        _CACHED[key] = builders[VERSION](T)
    nc = _CACHED[key]
    if VERSION in ("v5", "v6", "v7", "v8", "v9", "v10", "v11"):
        in_maps = [{"emb16": np.ascontiguousarray(
                        node_embedding[b].reshape(2, 128, F)
                        .transpose(1, 0, 2)).astype(np.float16),
                    **preps[VERSION](nbr16[b], T)}
                   for b in range(B)]
    else:
        in_maps = [{"emb": node_embedding[b], **preps[VERSION](nbr16[b], T)}
                   for b in range(B)]

    res = _run(nc, in_maps)
    if _collect is not None:
        _collect.append(res)
    outs = [res.results[b]["out"].reshape(At, Nbr, Nbr, F) for b in range(B)]
    return np.stack(outs, axis=0)
